# revision 38
# baseline (speedup 1.0000x reference)
"""GPT MHA (RoPE, causal) on 8 TRN2 NeuronCores.

Sharding: core c = (batch b = c//2) x (head-group g = c%2, 8 heads each).
Each core: Q/K/V projections for its 8 heads (column shards of Wq/Wk/Wv),
attention, and a row-shard out-projection producing a partial (S, E) fp32
output; the host sums the two partials per batch (row-parallel unshard).

Device math is bf16 with fp32 PSUM accumulation, except the softmax
numerator pt = exp(logit - 2) which is stored fp8(e4m3): measured noise
contribution ~1.2e-2 rel (gate 2e-2), and it halves the PE cost of the
softmax-z accumulation via a DoubleRow fp8 matmul over k-tile pairs
(ones-vector stationary). PV consumes the same fp8 pt as the moving
operand against bf16 V (mixed-dtype matmul) so no second pt copy exists.
Host-side prep folds 1/sqrt(d) into Wq and applies a per-head column
permutation to Wq/Wk so RoPE becomes contiguous block ops on chip.

Layout/scheduling notes (tuned against NTFF hardware profiles):
- K^T, V, and Q all stay resident in SBUF after projection (no DRAM
  round trips). fp8 pt tiles shrink phase-2 SBUF enough to hold Q.
- Projection weight tiles double-buffered (wbufs=2) so the next
  projection's weight DMA overlaps the current one's matmuls; the first
  V-proj tiles fetch fine-grained (startup is DMA-latency-bound).
- Attention emits ONE exp ACTIVATE per k-tile pair (reads a 2-bank PSUM
  tile, writes fp8): the per-call 352-cycle overhead otherwise makes
  the scalar engine the attention-rate limiter.
- Out-projection groups of chunk qc interleave 2-per-head into chunk
  qc-1's attention, emitted BETWEEN the first pair's QK/exp and its PV:
  at a head start the PE otherwise idles ~0.8us on that exp (mid-head
  the previous pair's PV covers the latency). They allocate from the
  pct pool (a QK-pair bank would still be waiting on its exp).
- ct leaves PSUM unnormalized right after the last PV (no z wait);
  normalization is an in-place SBUF mul off the critical path.
- Causal diagonal blocks stream only their unmasked columns; the z/exp
  pairing widens the two diagonal k-tiles of each chunk to their pair
  partner's width (band2-masked, both slots in one DVE mul), but PV
  still streams each k-tile's natural width.
- Softmax: z accumulated on the PE (fp8 DoubleRow over k-tile pairs,
  PSUM group per head); 1/z via reciprocal_approx_fast (DVE) +
  partition_broadcast (Pool); normalization fused into the PSUM->SBUF
  context copy.
- PSUM: phase 1 pmm bufs=5; phase 2 pmm2 2x[128,2,SC] for QK pairs
  (4 banks) + pct 3 (ct accumulators and out-proj groups) + z row.
- DMA: weights/activations fetched via grouped multi-dim descriptors on
  both HWDGE rings (sync+scalar) interleaved; wo on the sync ring only
  (the scalar queue feeds attention exps); output stored bf16, host
  sums partials in f32.
"""
import sys
import numpy as np

sys.path.insert(0, "/opt/trn_rl_repo")

import ml_dtypes

BF = ml_dtypes.bfloat16
F8 = ml_dtypes.float8_e4m3

B, S, E = 4, 2048, 2048
H, KS = 16, 128
HG = 8              # heads per core
D = HG * KS         # 1024 projected dims per core
ROT = 64
SC = 512            # s/q chunk
NSC = S // SC       # 4
NET = E // 128      # 16 e-tiles
NKT = S // 128      # 16 k-tiles
BAND_W = 896        # mask band table width
C_EXP = 2.0         # exp centering: pt = exp(logit - C_EXP), z cancels it

_PROG = {}


def _build_program(wbufs=2):
    import concourse.bass as bass
    import concourse.tile as tile
    import concourse.mybir as mybir
    from concourse import bacc
    from concourse.bass import ts, ds
    from contextlib import ExitStack

    f32 = mybir.dt.float32
    bf16 = mybir.dt.bfloat16
    fp8 = mybir.dt.float8e4
    AF = mybir.ActivationFunctionType
    DR = mybir.MatmulPerfMode.DoubleRow

    nc = bacc.Bacc("TRN2", target_bir_lowering=False, debug=False, num_devices=8)

    xq_d = nc.dram_tensor("xqT", [E, S], bf16, kind="ExternalInput").ap()
    xk_d = nc.dram_tensor("xkT", [E, S], bf16, kind="ExternalInput").ap()
    xv_d = nc.dram_tensor("xvT", [E, S], bf16, kind="ExternalInput").ap()
    wq_d = nc.dram_tensor("wq", [E, D], bf16, kind="ExternalInput").ap()
    wk_d = nc.dram_tensor("wk", [E, D], bf16, kind="ExternalInput").ap()
    wv_d = nc.dram_tensor("wv", [E, D], bf16, kind="ExternalInput").ap()
    wo_d = nc.dram_tensor("wo", [D, E], bf16, kind="ExternalInput").ap()
    ctab_d = nc.dram_tensor("ctab", [128, S], bf16, kind="ExternalInput").ap()
    stab_d = nc.dram_tensor("stab", [64, S], bf16, kind="ExternalInput").ap()
    band_d = nc.dram_tensor("band", [128, 2 * SC], fp8, kind="ExternalInput").ap()
    out_d = nc.dram_tensor("out", [S, E], bf16, kind="ExternalOutput").ap()

    with tile.TileContext(nc) as tc, ExitStack() as ctx:
        const = ctx.enter_context(tc.tile_pool(name="const", bufs=1))
        persist = ctx.enter_context(tc.tile_pool(name="persist", bufs=1))

        ctab = const.tile([128, S], bf16)
        stab = const.tile([64, S], bf16)
        band2 = const.tile([128, 2, SC], fp8)   # paired diag masks (ext 0/128)
        ones2 = const.tile([128, 2, 16], fp8)   # z DoubleRow stationary
        expb = const.tile([128, 1], f32)        # exp bias (-C_EXP)

        v_all = persist.tile([128, NKT, D], bf16)     # V[s, d] per k-tile
        k_all = persist.tile([128, HG, S], bf16)      # K^T[d, k] per head
        q_all = persist.tile([128, HG, S], bf16)      # Q^T[d, q] per head

        nc.vector.memset(ones2[:], 1.0)
        nc.vector.memset(expb[:], -C_EXP)

        # ---------------- Phase 1: projections (+RoPE for Q/K) ------------
        with tc.tile_pool(name="wpool", bufs=wbufs) as wpool, \
                tc.tile_pool(name="xtp", bufs=2) as xtp, \
                tc.tile_pool(name="rope", bufs=2) as rope, \
                tc.tile_pool(name="pmm", bufs=5, space="PSUM") as pmm:
            for proj, w_src, x_src in (("v", wv_d, xv_d), ("k", wk_d, xk_d),
                                       ("q", wq_d, xq_d)):
                w_sb = wpool.tile([128, NET, D], bf16, tag="wt")
                xt0 = xtp.tile([128, NET, SC], bf16, tag="xt")
                s0 = 3 if proj == "q" else 0      # first-emitted chunk
                if proj == "v":
                    # startup is DMA-latency-bound: fully fine-grained
                    # fetches keep the first matmul chain fed e-tile by
                    # e-tile as transfers land
                    for et in range(NET):
                        weng = nc.scalar if et % 2 == 0 else nc.sync
                        xeng = nc.sync if et % 2 == 0 else nc.scalar
                        weng.dma_start(w_sb[:, et, :],
                                       w_src[ts(et, 128), :])
                        xeng.dma_start(xt0[:, et, :],
                                       x_src[ts(et, 128), 0:SC])
                else:
                    # transition: fine-grained so the first chain starts fast
                    for et in range(NET):
                        weng = nc.scalar if et % 2 == 0 else nc.sync
                        xeng = nc.sync if et % 2 == 0 else nc.scalar
                        weng.dma_start(w_sb[:, et, :],
                                       w_src[ts(et, 128), :])
                        xeng.dma_start(xt0[:, et, :],
                                       x_src[ts(et, 128), ts(s0, SC)])
                if proj == "k":
                    # Deferred prefetch: tables are first needed by K's RoPE;
                    # keep them off the rings during V-proj.
                    nc.scalar.dma_start(ctab[:], ctab_d[:])
                    nc.scalar.dma_start(stab[:], stab_d[:])
                    nc.scalar.dma_start(band2[:], band_d[:])
                sc_order = [3, 2, 1, 0] if proj == "q" else [0, 1, 2, 3]
                for si, sc in enumerate(sc_order):
                    if si == 0:
                        xt = xt0
                    else:
                        xt = xtp.tile([128, NET, SC], bf16, tag="xt")
                        for eg in range(4):
                            eng = nc.sync if eg % 2 == 0 else nc.scalar
                            eng.dma_start(
                                xt[:, ts(eg, 4), :],
                                x_src[ts(eg, 512), ts(sc, SC)].rearrange(
                                    "(et p) s -> p et s", p=128))
                    if proj in ("q", "k"):
                        for h in range(HG):
                            ps = pmm.tile([128, SC], f32, tag="mm")
                            for et in range(NET):
                                nc.tensor.matmul(
                                    ps[:], w_sb[:, et, ts(h, 128)],
                                    xt[:, et, :],
                                    start=(et == 0), stop=(et == NET - 1))
                            qraw = rope.tile([128, SC], bf16, tag="qraw")
                            nc.vector.tensor_copy(qraw[:], ps[:])
                            qsw = rope.tile([64, SC], bf16, tag="qsw")
                            # one swap per ring: both on sync delays the
                            # x-fetches queued behind them
                            nc.sync.dma_start(qsw[0:32, :], qraw[32:64, :])
                            nc.scalar.dma_start(qsw[32:64, :], qraw[0:32, :])
                            if proj == "k":
                                dst = k_all[:, h, ts(sc, SC)]
                            else:
                                dst = q_all[:, h, ts(sc, SC)]
                            nc.vector.tensor_mul(dst, qraw[:],
                                                 ctab[:, ts(sc, SC)])
                            t2 = rope.tile([64, SC], bf16, tag="t2")
                            nc.vector.tensor_mul(t2[:], qsw[:],
                                                 stab[:, ts(sc, SC)])
                            nc.vector.tensor_add(dst[0:64, :], dst[0:64, :],
                                                 t2[:])
                    else:
                        for ss in range(SC // 128):
                            for dc in range(D // SC):
                                ps = pmm.tile([128, SC], f32, tag="mm")
                                for et in range(NET):
                                    nc.tensor.matmul(
                                        ps[:], xt[:, et, ts(ss, 128)],
                                        w_sb[:, et, ts(dc, SC)],
                                        start=(et == 0), stop=(et == NET - 1))
                                nc.vector.tensor_copy(
                                    v_all[:, sc * 4 + ss, ts(dc, SC)], ps[:])

        # -------- Phase 2: fused attention + out-projection per q-chunk ---
        p2 = ExitStack()
        ctx.enter_context(p2)
        wop = p2.enter_context(tc.tile_pool(name="wop", bufs=1))
        ptp = p2.enter_context(tc.tile_pool(name="ptp", bufs=12))
        ctsbp = p2.enter_context(tc.tile_pool(name="ctsbp", bufs=2))
        zp = p2.enter_context(tc.tile_pool(name="zp", bufs=3))
        rbp = p2.enter_context(tc.tile_pool(name="rbp", bufs=3))
        osb = p2.enter_context(tc.tile_pool(name="osb", bufs=3))
        # PSUM: pmm2 holds QK pairs AND out-proj accumulators (4 banks),
        # pct the per-head context accumulators (3), pz the z rows (1).
        pmm2 = p2.enter_context(tc.tile_pool(name="pmm2", bufs=2,
                                             space="PSUM"))
        pct = p2.enter_context(tc.tile_pool(name="pct", bufs=3, space="PSUM"))
        pz = p2.enter_context(tc.tile_pool(name="pz", bufs=1, space="PSUM"))
        wo_sb = wop.tile([128, HG, E], bf16)          # Wo rows per head
        for g in range(HG):
            # sync ring only: the scalar queue feeds attention exps now
            nc.sync.dma_start(wo_sb[:, g, :], wo_d[ts(g, 128), :])
        zt = pz.tile([1, SC], f32)                    # z row (partition 0)

        def outproj_group(qc, ct_sb, qt, ec):
            # allocate from pct: avoids stealing a QK-pair bank whose exp
            # is still pending at head boundaries
            o_ps = pct.tile([128, SC], f32, tag="ctps")
            for hh in range(HG):
                nc.tensor.matmul(o_ps[:],
                                 ct_sb[:, hh, ts(qt, 128)],
                                 wo_sb[:, hh, ts(ec, SC)],
                                 start=(hh == 0), stop=(hh == HG - 1))
            o_sb = osb.tile([128, SC], bf16, tag="o")
            nc.vector.tensor_copy(o_sb[:], o_ps[:])
            # sync ring only: a scalar-ring store would sit ahead of the
            # next exp ACTIVATE (~600ns queue time each) in the attention
            # stretch these groups interleave into
            nc.sync.dma_start(out_d[ts(qc * 4 + qt, 128), ts(ec, SC)],
                              o_sb[:])

        def outproj_groups(qc, ct_sb):
            # 16 (qt, ec) groups; interleaved 2-per-head into the NEXT
            # chunk's attention so the PE absorbs the scalar exp rate and
            # the scalar queue never drains during out-projection.
            for qt in range(SC // 128):
                for ec in range(E // SC):
                    yield (qc, ct_sb, qt, ec)

        prev = None
        for qc in (3, 2, 1, 0):
            nkt = 4 * qc + 4
            ct_sb = ctsbp.tile([128, HG, SC], bf16, tag="ct")
            for h in range(HG):
                ct_ps = pct.tile([128, SC], f32, tag="ctps")
                z_ps = zt[:]
                for p in range(nkt // 2):
                    kt0 = 2 * p
                    off0 = kt0 * 128 - qc * SC
                    # pair window: both slots span [cs, cs+w)
                    cs = max(0, off0)
                    w = SC - cs
                    pt2 = ptp.tile([128, 2, SC], fp8, tag="pt")
                    l2 = pmm2.tile([128, 2, SC], f32, tag="mm2")
                    for j in range(2):
                        nc.tensor.matmul(l2[:, j, 0:w],
                                         k_all[:, h, ts(kt0 + j, 128)],
                                         q_all[:, h, ds(qc * SC + cs, w)],
                                         start=True, stop=True)
                    # one exp per pair: reads both PSUM banks, writes fp8
                    nc.scalar.activation(pt2[:, :, 0:w], l2[:, :, 0:w],
                                         AF.Exp, bias=expb[:])
                    if off0 + 128 > 0:
                        # diagonal pair: slots always mask at ext (0, 128)
                        nc.vector.tensor_mul(pt2[:, :, 0:w], pt2[:, :, 0:w],
                                             band2[:, :, 0:w])
                    if p == 0 and prev is not None:
                        # fill the head-start exp-latency bubble (the first
                        # PV waits ~1us on this pair's exp; mid-head the
                        # previous pair's PV covers it) with out-proj work
                        for _ in range(2):
                            g = next(prev, None)
                            if g is not None:
                                outproj_group(*g)
                    for j in range(2):
                        kt = kt0 + j
                        pcs = max(0, kt * 128 - qc * SC)
                        nc.tensor.matmul(
                            ct_ps[:, ds(pcs, SC - pcs)],
                            v_all[:, kt, ts(h, 128)],
                            pt2[:, j, ds(pcs - cs, SC - pcs)],
                            start=(kt == 0), stop=(kt == nkt - 1),
                            skip_group_check=True)
                    nc.tensor.matmul(z_ps[:, ds(cs, w)], ones2[:, :, 0:1],
                                     pt2[:, :, 0:w],
                                     start=(p == 0), stop=(p == nkt // 2 - 1),
                                     perf_mode=DR, skip_group_check=True)
                # Copy ct out UNNORMALIZED right after the last PV: frees the
                # PSUM bank without waiting the z->recip->broadcast chain
                # (otherwise the next head's first PV stalls ~0.8us on the
                # bank). Normalization happens in-place in SBUF off the
                # critical path; out-proj depends on the in-place mul.
                nc.vector.tensor_copy(ct_sb[:, h, :], ct_ps[:])
                zr = zp.tile([1, SC], f32, tag="zr")
                nc.vector.reciprocal_approx_fast(zr[:], z_ps)
                rb = rbp.tile([128, SC], f32, tag="rb")
                nc.gpsimd.partition_broadcast(rb[:], zr[:])
                nc.vector.tensor_mul(ct_sb[:, h, :], ct_sb[:, h, :], rb[:])
            prev = outproj_groups(qc, ct_sb)
        for g in prev:
            outproj_group(*g)

    nc.compile()
    return nc


def _get_program():
    if "nc" not in _PROG:
        try:
            _PROG["nc"] = _build_program(wbufs=2)
        except Exception:
            _PROG["nc"] = _build_program(wbufs=1)
    return _PROG["nc"]


def _host_prep(query_inputs, key_inputs, value_inputs, Wq, Wk, Wv, Wo):
    """Shard + bf16-cast inputs; fold scale/permutation into Wq/Wk."""
    perm = np.concatenate([np.arange(0, ROT, 2), np.arange(1, ROT, 2),
                           np.arange(ROT, KS)])
    Wq_p = (Wq.reshape(E, H, KS)[:, :, perm] / np.float32(np.sqrt(KS))
            ).reshape(E, H * KS)
    Wk_p = Wk.reshape(E, H, KS)[:, :, perm].reshape(E, H * KS)

    inv_freq = 1.0 / (10000.0 ** (np.arange(0, ROT, 2, dtype=np.float32) / ROT))
    ang = np.outer(np.arange(S, dtype=np.float32), inv_freq)  # (S, 32)
    sin = np.sin(ang).T.astype(np.float32)
    cos = np.cos(ang).T.astype(np.float32)
    ctab = np.ones((128, S), np.float32)
    ctab[0:32] = cos
    ctab[32:64] = cos
    stab = np.zeros((64, S), np.float32)
    stab[0:32] = -sin
    stab[32:64] = sin
    # band[i, c] = 1 iff (c - 384) >= i. Diagonal pairs always mask their
    # two k-tile slots at ext (0, 128): band2 slot0 = band[:, 384:896],
    # slot1 = band[:, 256:768], each 512 wide (use first w cols).
    cgrid = np.arange(BAND_W)[None, :] - 384
    band = (cgrid >= np.arange(128)[:, None]).astype(np.float32)
    band2 = np.concatenate([band[:, 384:896], band[:, 256:768]], axis=1)

    shared = {
        "ctab": ctab.astype(BF),
        "stab": stab.astype(BF),
        "band": band2.astype(F8),
    }
    in_maps = []
    for c in range(8):
        b, g = c // 2, c % 2
        cols = slice(g * D, (g + 1) * D)
        in_maps.append({
            "xqT": np.ascontiguousarray(query_inputs[b].T).astype(BF),
            "xkT": np.ascontiguousarray(key_inputs[b].T).astype(BF),
            "xvT": np.ascontiguousarray(value_inputs[b].T).astype(BF),
            "wq": np.ascontiguousarray(Wq_p[:, cols]).astype(BF),
            "wk": np.ascontiguousarray(Wk_p[:, cols]).astype(BF),
            "wv": np.ascontiguousarray(Wv[:, cols]).astype(BF),
            "wo": np.ascontiguousarray(Wo[cols, :]).astype(BF),
            **shared,
        })
    return in_maps


def run_sharded(inputs, trace=False, **trace_kw):
    """Build + run the SPMD kernel; returns (output, BassKernelResults)."""
    from concourse.bass_utils import run_bass_kernel_spmd

    nc = _get_program()
    in_maps = _host_prep(
        np.asarray(inputs["query_inputs"], np.float32),
        np.asarray(inputs["key_inputs"], np.float32),
        np.asarray(inputs["value_inputs"], np.float32),
        np.asarray(inputs["Wq"], np.float32),
        np.asarray(inputs["Wk"], np.float32),
        np.asarray(inputs["Wv"], np.float32),
        np.asarray(inputs["Wo"], np.float32),
    )
    br = run_bass_kernel_spmd(nc, in_maps, list(range(8)), trace=trace,
                              **trace_kw)
    parts = [np.asarray(r["out"], np.float32) for r in br.results]
    out = np.stack([parts[2 * b] + parts[2 * b + 1] for b in range(B)])
    return out, br


def kernel(query_inputs, key_inputs, value_inputs, attention_mask,
           Wq, Wk, Wv, Wo):
    out, _ = run_sharded({
        "query_inputs": query_inputs, "key_inputs": key_inputs,
        "value_inputs": value_inputs, "Wq": Wq, "Wk": Wk, "Wv": Wv, "Wo": Wo,
    })
    return out


# revision 41
# speedup vs baseline: 1.0433x; 1.0433x over previous
"""GPT MHA (RoPE, causal) on 8 TRN2 NeuronCores.

Sharding: core c = (batch b = c//2) x (head-group g = c%2, 8 heads each).
Each core: Q/K/V projections for its 8 heads (column shards of Wq/Wk/Wv),
attention, and a row-shard out-projection producing a partial (S, E) fp32
output; the host sums the two partials per batch (row-parallel unshard).

Device math is bf16 with fp32 PSUM accumulation, except the softmax
numerator pt = exp(logit - 2) which is stored fp8(e4m3): measured noise
contribution ~1.2e-2 rel (gate 2e-2), and it halves the PE cost of the
softmax-z accumulation via a DoubleRow fp8 matmul over k-tile pairs
(ones-vector stationary). PV consumes the same fp8 pt as the moving
operand against bf16 V (mixed-dtype matmul) so no second pt copy exists.
Host-side prep folds 1/sqrt(d) into Wq and applies a per-head column
permutation to Wq/Wk so RoPE becomes contiguous block ops on chip.

Layout/scheduling notes (tuned against NTFF hardware profiles):
- K^T, V, and Q all stay resident in SBUF after projection (no DRAM
  round trips). fp8 pt tiles shrink phase-2 SBUF enough to hold Q.
- Projection weight tiles double-buffered (wbufs=2) so the next
  projection's weight DMA overlaps the current one's matmuls; the first
  V-proj tiles fetch fine-grained (startup is DMA-latency-bound).
- Attention emits ONE exp ACTIVATE per k-tile pair (reads a 2-bank PSUM
  tile, writes fp8): the per-call 352-cycle overhead otherwise makes
  the scalar engine the attention-rate limiter.
- Out-projection groups of chunk qc interleave 2-per-head into chunk
  qc-1's attention, emitted BETWEEN the first pair's QK/exp and its PV:
  at a head start the PE otherwise idles ~0.8us on that exp (mid-head
  the previous pair's PV covers the latency). They allocate from the
  pct pool (a QK-pair bank would still be waiting on its exp).
- ct leaves PSUM unnormalized right after the last PV (no z wait);
  normalization is an in-place SBUF mul off the critical path.
- Causal diagonal blocks stream only their unmasked columns; the z/exp
  pairing widens the two diagonal k-tiles of each chunk to their pair
  partner's width (band2-masked, both slots in one DVE mul), but PV
  still streams each k-tile's natural width.
- Softmax: z accumulated on the PE (fp8 DoubleRow over k-tile pairs,
  PSUM group per head); 1/z via reciprocal_approx_fast (DVE) +
  partition_broadcast (Pool); normalization fused into the PSUM->SBUF
  context copy.
- PSUM: phase 1 pmm bufs=5; phase 2 pmm2 2x[128,2,SC] for QK pairs
  (4 banks) + pct 3 (ct accumulators and out-proj groups) + z row.
- DMA: weights/activations fetched via grouped multi-dim descriptors on
  both HWDGE rings (sync+scalar) interleaved; wo on the sync ring only
  (the scalar queue feeds attention exps); output stored bf16, host
  sums partials in f32.
"""
import sys
import numpy as np

sys.path.insert(0, "/opt/trn_rl_repo")

import ml_dtypes

BF = ml_dtypes.bfloat16
F8 = ml_dtypes.float8_e4m3

B, S, E = 4, 2048, 2048
H, KS = 16, 128
HG = 8              # heads per core
D = HG * KS         # 1024 projected dims per core
ROT = 64
SC = 512            # s/q chunk
NSC = S // SC       # 4
NET = E // 128      # 16 e-tiles
NKT = S // 128      # 16 k-tiles
BAND_W = 896        # mask band table width
C_EXP = 2.0         # exp centering: pt = exp(logit - C_EXP), z cancels it

_PROG = {}


def _build_program(wbufs=2):
    import concourse.bass as bass
    import concourse.tile as tile
    import concourse.mybir as mybir
    from concourse import bacc
    from concourse.bass import ts, ds
    from contextlib import ExitStack

    f32 = mybir.dt.float32
    bf16 = mybir.dt.bfloat16
    fp8 = mybir.dt.float8e4
    AF = mybir.ActivationFunctionType
    DR = mybir.MatmulPerfMode.DoubleRow

    nc = bacc.Bacc("TRN2", target_bir_lowering=False, debug=False, num_devices=8)

    xq_d = nc.dram_tensor("xqT", [E, S], bf16, kind="ExternalInput").ap()
    xk_d = nc.dram_tensor("xkT", [E, S], bf16, kind="ExternalInput").ap()
    xv_d = nc.dram_tensor("xvT", [E, S], bf16, kind="ExternalInput").ap()
    wq_d = nc.dram_tensor("wq", [E, D], bf16, kind="ExternalInput").ap()
    wk_d = nc.dram_tensor("wk", [E, D], bf16, kind="ExternalInput").ap()
    wv_d = nc.dram_tensor("wv", [E, D], bf16, kind="ExternalInput").ap()
    wo_d = nc.dram_tensor("wo", [D, E], bf16, kind="ExternalInput").ap()
    ctab_d = nc.dram_tensor("ctab", [128, S], bf16, kind="ExternalInput").ap()
    stab_d = nc.dram_tensor("stab", [64, S], bf16, kind="ExternalInput").ap()
    band_d = nc.dram_tensor("band", [128, 2 * SC], fp8, kind="ExternalInput").ap()
    out_d = nc.dram_tensor("out", [S, E], bf16, kind="ExternalOutput").ap()

    with tile.TileContext(nc) as tc, ExitStack() as ctx:
        const = ctx.enter_context(tc.tile_pool(name="const", bufs=1))
        persist = ctx.enter_context(tc.tile_pool(name="persist", bufs=1))

        ctab = const.tile([128, S], bf16)
        stab = const.tile([64, S], bf16)
        band2 = const.tile([128, 2, SC], fp8)   # paired diag masks (ext 0/128)
        ones2 = const.tile([128, 2, 16], fp8)   # z DoubleRow stationary
        expb = const.tile([128, 1], f32)        # exp bias (-C_EXP)

        v_all = persist.tile([128, NKT, D], bf16)     # V[s, d] per k-tile
        k_all = persist.tile([128, HG, S], bf16)      # K^T[d, k] per head
        q_all = persist.tile([128, HG, S], bf16)      # Q^T[d, q] per head

        nc.vector.memset(ones2[:], 1.0)
        nc.vector.memset(expb[:], -C_EXP)

        # ---------------- Phase 1: projections (+RoPE for Q/K) ------------
        with tc.tile_pool(name="wpool", bufs=wbufs) as wpool, \
                tc.tile_pool(name="xtp", bufs=2) as xtp, \
                tc.tile_pool(name="rope", bufs=2) as rope, \
                tc.tile_pool(name="pmm", bufs=5, space="PSUM") as pmm:
            for proj, w_src, x_src in (("v", wv_d, xv_d), ("k", wk_d, xk_d),
                                       ("q", wq_d, xq_d)):
                w_sb = wpool.tile([128, NET, D], bf16, tag="wt")
                xt0 = xtp.tile([128, NET, SC], bf16, tag="xt")
                s0 = 3 if proj == "q" else 0      # first-emitted chunk
                if proj == "v":
                    # startup is DMA-latency-bound: fully fine-grained
                    # fetches, weights across BOTH HWDGE rings and x on the
                    # (otherwise idle) vector DGE as a third channel
                    for et in range(NET):
                        weng = nc.scalar if et % 2 == 0 else nc.sync
                        weng.dma_start(w_sb[:, et, :],
                                       w_src[ts(et, 128), :])
                        nc.gpsimd.dma_start(xt0[:, et, :],
                                            x_src[ts(et, 128), 0:SC])
                else:
                    # transition: fine-grained so the first chain starts fast
                    for et in range(NET):
                        weng = nc.scalar if et % 2 == 0 else nc.sync
                        xeng = nc.sync if et % 2 == 0 else nc.scalar
                        weng.dma_start(w_sb[:, et, :],
                                       w_src[ts(et, 128), :])
                        xeng.dma_start(xt0[:, et, :],
                                       x_src[ts(et, 128), ts(s0, SC)])
                if proj == "k":
                    # Deferred prefetch: tables are first needed by K's RoPE;
                    # keep them off the rings during V-proj.
                    nc.scalar.dma_start(ctab[:], ctab_d[:])
                    nc.scalar.dma_start(stab[:], stab_d[:])
                    nc.scalar.dma_start(band2[:], band_d[:])
                sc_order = [3, 2, 1, 0] if proj == "q" else [0, 1, 2, 3]
                for si, sc in enumerate(sc_order):
                    if si == 0:
                        xt = xt0
                    else:
                        xt = xtp.tile([128, NET, SC], bf16, tag="xt")
                        for eg in range(4):
                            eng = nc.sync if eg % 2 == 0 else nc.scalar
                            eng.dma_start(
                                xt[:, ts(eg, 4), :],
                                x_src[ts(eg, 512), ts(sc, SC)].rearrange(
                                    "(et p) s -> p et s", p=128))
                    if proj in ("q", "k"):
                        for h in range(HG):
                            ps = pmm.tile([128, SC], f32, tag="mm")
                            for et in range(NET):
                                nc.tensor.matmul(
                                    ps[:], w_sb[:, et, ts(h, 128)],
                                    xt[:, et, :],
                                    start=(et == 0), stop=(et == NET - 1))
                            qraw = rope.tile([128, SC], bf16, tag="qraw")
                            nc.vector.tensor_copy(qraw[:], ps[:])
                            qsw = rope.tile([64, SC], bf16, tag="qsw")
                            nc.sync.dma_start(qsw[0:32, :], qraw[32:64, :])
                            nc.sync.dma_start(qsw[32:64, :], qraw[0:32, :])
                            if proj == "k":
                                dst = k_all[:, h, ts(sc, SC)]
                            else:
                                dst = q_all[:, h, ts(sc, SC)]
                            nc.vector.tensor_mul(dst, qraw[:],
                                                 ctab[:, ts(sc, SC)])
                            t2 = rope.tile([64, SC], bf16, tag="t2")
                            nc.vector.tensor_mul(t2[:], qsw[:],
                                                 stab[:, ts(sc, SC)])
                            nc.vector.tensor_add(dst[0:64, :], dst[0:64, :],
                                                 t2[:])
                    else:
                        for ss in range(SC // 128):
                            for dc in range(D // SC):
                                ps = pmm.tile([128, SC], f32, tag="mm")
                                for et in range(NET):
                                    nc.tensor.matmul(
                                        ps[:], xt[:, et, ts(ss, 128)],
                                        w_sb[:, et, ts(dc, SC)],
                                        start=(et == 0), stop=(et == NET - 1))
                                nc.vector.tensor_copy(
                                    v_all[:, sc * 4 + ss, ts(dc, SC)], ps[:])

        # -------- Phase 2: fused attention + out-projection per q-chunk ---
        p2 = ExitStack()
        ctx.enter_context(p2)
        wop = p2.enter_context(tc.tile_pool(name="wop", bufs=1))
        ptp = p2.enter_context(tc.tile_pool(name="ptp", bufs=12))
        ctsbp = p2.enter_context(tc.tile_pool(name="ctsbp", bufs=2))
        zp = p2.enter_context(tc.tile_pool(name="zp", bufs=3))
        rbp = p2.enter_context(tc.tile_pool(name="rbp", bufs=3))
        osb = p2.enter_context(tc.tile_pool(name="osb", bufs=3))
        # PSUM: pmm2 holds QK pairs AND out-proj accumulators (4 banks),
        # pct the per-head context accumulators (3), pz the z rows (1).
        pmm2 = p2.enter_context(tc.tile_pool(name="pmm2", bufs=2,
                                             space="PSUM"))
        pct = p2.enter_context(tc.tile_pool(name="pct", bufs=3, space="PSUM"))
        pz = p2.enter_context(tc.tile_pool(name="pz", bufs=1, space="PSUM"))
        wo_sb = wop.tile([128, HG, E], bf16)          # Wo rows per head
        for g in range(HG):
            # sync ring only: the scalar queue feeds attention exps now
            nc.sync.dma_start(wo_sb[:, g, :], wo_d[ts(g, 128), :])
        zt = pz.tile([1, SC], f32)                    # z row (partition 0)

        def outproj_group(qc, ct_sb, qt, ec):
            # allocate from pct: avoids stealing a QK-pair bank whose exp
            # is still pending at head boundaries
            o_ps = pct.tile([128, SC], f32, tag="ctps")
            for hh in range(HG):
                nc.tensor.matmul(o_ps[:],
                                 ct_sb[:, hh, ts(qt, 128)],
                                 wo_sb[:, hh, ts(ec, SC)],
                                 start=(hh == 0), stop=(hh == HG - 1))
            o_sb = osb.tile([128, SC], bf16, tag="o")
            nc.vector.tensor_copy(o_sb[:], o_ps[:])
            eng = nc.sync if (qt + ec) % 2 == 0 else nc.scalar
            eng.dma_start(out_d[ts(qc * 4 + qt, 128), ts(ec, SC)], o_sb[:])

        def outproj_groups(qc, ct_sb):
            # 16 (qt, ec) groups; interleaved 2-per-head into the NEXT
            # chunk's attention so the PE absorbs the scalar exp rate and
            # the scalar queue never drains during out-projection.
            for qt in range(SC // 128):
                for ec in range(E // SC):
                    yield (qc, ct_sb, qt, ec)

        prev = None
        for qc in (3, 2, 1, 0):
            nkt = 4 * qc + 4
            ct_sb = ctsbp.tile([128, HG, SC], bf16, tag="ct")
            for h in range(HG):
                ct_ps = pct.tile([128, SC], f32, tag="ctps")
                z_ps = zt[:]
                for p in range(nkt // 2):
                    kt0 = 2 * p
                    off0 = kt0 * 128 - qc * SC
                    # pair window: both slots span [cs, cs+w)
                    cs = max(0, off0)
                    w = SC - cs
                    pt2 = ptp.tile([128, 2, SC], fp8, tag="pt")
                    l2 = pmm2.tile([128, 2, SC], f32, tag="mm2")
                    for j in range(2):
                        nc.tensor.matmul(l2[:, j, 0:w],
                                         k_all[:, h, ts(kt0 + j, 128)],
                                         q_all[:, h, ds(qc * SC + cs, w)],
                                         start=True, stop=True)
                    # one exp per pair: reads both PSUM banks, writes fp8
                    nc.scalar.activation(pt2[:, :, 0:w], l2[:, :, 0:w],
                                         AF.Exp, bias=expb[:])
                    if off0 + 128 > 0:
                        # diagonal pair: slots always mask at ext (0, 128)
                        nc.vector.tensor_mul(pt2[:, :, 0:w], pt2[:, :, 0:w],
                                             band2[:, :, 0:w])
                    if p == 0 and prev is not None:
                        # fill the head-start exp-latency bubble (the first
                        # PV waits ~1us on this pair's exp; mid-head the
                        # previous pair's PV covers it) with out-proj work
                        for _ in range(2):
                            g = next(prev, None)
                            if g is not None:
                                outproj_group(*g)
                    for j in range(2):
                        kt = kt0 + j
                        pcs = max(0, kt * 128 - qc * SC)
                        nc.tensor.matmul(
                            ct_ps[:, ds(pcs, SC - pcs)],
                            v_all[:, kt, ts(h, 128)],
                            pt2[:, j, ds(pcs - cs, SC - pcs)],
                            start=(kt == 0), stop=(kt == nkt - 1),
                            skip_group_check=True)
                    nc.tensor.matmul(z_ps[:, ds(cs, w)], ones2[:, :, 0:1],
                                     pt2[:, :, 0:w],
                                     start=(p == 0), stop=(p == nkt // 2 - 1),
                                     perf_mode=DR, skip_group_check=True)
                # Copy ct out UNNORMALIZED right after the last PV: frees the
                # PSUM bank without waiting the z->recip->broadcast chain
                # (otherwise the next head's first PV stalls ~0.8us on the
                # bank). Normalization happens in-place in SBUF off the
                # critical path; out-proj depends on the in-place mul.
                nc.vector.tensor_copy(ct_sb[:, h, :], ct_ps[:])
                zr = zp.tile([1, SC], f32, tag="zr")
                nc.vector.reciprocal_approx_fast(zr[:], z_ps)
                rb = rbp.tile([128, SC], f32, tag="rb")
                nc.gpsimd.partition_broadcast(rb[:], zr[:])
                nc.vector.tensor_mul(ct_sb[:, h, :], ct_sb[:, h, :], rb[:])
            prev = outproj_groups(qc, ct_sb)
        for g in prev:
            outproj_group(*g)

    nc.compile()
    return nc


def _get_program():
    if "nc" not in _PROG:
        try:
            _PROG["nc"] = _build_program(wbufs=2)
        except Exception:
            _PROG["nc"] = _build_program(wbufs=1)
    return _PROG["nc"]


def _host_prep(query_inputs, key_inputs, value_inputs, Wq, Wk, Wv, Wo):
    """Shard + bf16-cast inputs; fold scale/permutation into Wq/Wk."""
    perm = np.concatenate([np.arange(0, ROT, 2), np.arange(1, ROT, 2),
                           np.arange(ROT, KS)])
    Wq_p = (Wq.reshape(E, H, KS)[:, :, perm] / np.float32(np.sqrt(KS))
            ).reshape(E, H * KS)
    Wk_p = Wk.reshape(E, H, KS)[:, :, perm].reshape(E, H * KS)

    inv_freq = 1.0 / (10000.0 ** (np.arange(0, ROT, 2, dtype=np.float32) / ROT))
    ang = np.outer(np.arange(S, dtype=np.float32), inv_freq)  # (S, 32)
    sin = np.sin(ang).T.astype(np.float32)
    cos = np.cos(ang).T.astype(np.float32)
    ctab = np.ones((128, S), np.float32)
    ctab[0:32] = cos
    ctab[32:64] = cos
    stab = np.zeros((64, S), np.float32)
    stab[0:32] = -sin
    stab[32:64] = sin
    # band[i, c] = 1 iff (c - 384) >= i. Diagonal pairs always mask their
    # two k-tile slots at ext (0, 128): band2 slot0 = band[:, 384:896],
    # slot1 = band[:, 256:768], each 512 wide (use first w cols).
    cgrid = np.arange(BAND_W)[None, :] - 384
    band = (cgrid >= np.arange(128)[:, None]).astype(np.float32)
    band2 = np.concatenate([band[:, 384:896], band[:, 256:768]], axis=1)

    shared = {
        "ctab": ctab.astype(BF),
        "stab": stab.astype(BF),
        "band": band2.astype(F8),
    }
    in_maps = []
    for c in range(8):
        b, g = c // 2, c % 2
        cols = slice(g * D, (g + 1) * D)
        in_maps.append({
            "xqT": np.ascontiguousarray(query_inputs[b].T).astype(BF),
            "xkT": np.ascontiguousarray(key_inputs[b].T).astype(BF),
            "xvT": np.ascontiguousarray(value_inputs[b].T).astype(BF),
            "wq": np.ascontiguousarray(Wq_p[:, cols]).astype(BF),
            "wk": np.ascontiguousarray(Wk_p[:, cols]).astype(BF),
            "wv": np.ascontiguousarray(Wv[:, cols]).astype(BF),
            "wo": np.ascontiguousarray(Wo[cols, :]).astype(BF),
            **shared,
        })
    return in_maps


def run_sharded(inputs, trace=False, **trace_kw):
    """Build + run the SPMD kernel; returns (output, BassKernelResults)."""
    from concourse.bass_utils import run_bass_kernel_spmd

    nc = _get_program()
    in_maps = _host_prep(
        np.asarray(inputs["query_inputs"], np.float32),
        np.asarray(inputs["key_inputs"], np.float32),
        np.asarray(inputs["value_inputs"], np.float32),
        np.asarray(inputs["Wq"], np.float32),
        np.asarray(inputs["Wk"], np.float32),
        np.asarray(inputs["Wv"], np.float32),
        np.asarray(inputs["Wo"], np.float32),
    )
    br = run_bass_kernel_spmd(nc, in_maps, list(range(8)), trace=trace,
                              **trace_kw)
    parts = [np.asarray(r["out"], np.float32) for r in br.results]
    out = np.stack([parts[2 * b] + parts[2 * b + 1] for b in range(B)])
    return out, br


def kernel(query_inputs, key_inputs, value_inputs, attention_mask,
           Wq, Wk, Wv, Wo):
    out, _ = run_sharded({
        "query_inputs": query_inputs, "key_inputs": key_inputs,
        "value_inputs": value_inputs, "Wq": Wq, "Wk": Wk, "Wv": Wv, "Wo": Wo,
    })
    return out


# revision 42
# speedup vs baseline: 1.0463x; 1.0029x over previous
"""GPT MHA (RoPE, causal) on 8 TRN2 NeuronCores.

Sharding: core c = (batch b = c//2) x (head-group g = c%2, 8 heads each).
Each core: Q/K/V projections for its 8 heads (column shards of Wq/Wk/Wv),
attention, and a row-shard out-projection producing a partial (S, E) fp32
output; the host sums the two partials per batch (row-parallel unshard).

Device math is bf16 with fp32 PSUM accumulation, except the softmax
numerator pt = exp(logit - 2) which is stored fp8(e4m3): measured noise
contribution ~1.2e-2 rel (gate 2e-2), and it halves the PE cost of the
softmax-z accumulation via a DoubleRow fp8 matmul over k-tile pairs
(ones-vector stationary). PV consumes the same fp8 pt as the moving
operand against bf16 V (mixed-dtype matmul) so no second pt copy exists.
Host-side prep folds 1/sqrt(d) into Wq and applies a per-head column
permutation to Wq/Wk so RoPE becomes contiguous block ops on chip.

Layout/scheduling notes (tuned against NTFF hardware profiles):
- K^T, V, and Q all stay resident in SBUF after projection (no DRAM
  round trips). fp8 pt tiles shrink phase-2 SBUF enough to hold Q.
- Projection weight tiles double-buffered (wbufs=2) so the next
  projection's weight DMA overlaps the current one's matmuls; the first
  V-proj tiles fetch fine-grained (startup is DMA-latency-bound).
- Attention emits ONE exp ACTIVATE per k-tile pair (reads a 2-bank PSUM
  tile, writes fp8): the per-call 352-cycle overhead otherwise makes
  the scalar engine the attention-rate limiter.
- Out-projection groups of chunk qc interleave 2-per-head into chunk
  qc-1's attention, emitted BETWEEN the first pair's QK/exp and its PV:
  at a head start the PE otherwise idles ~0.8us on that exp (mid-head
  the previous pair's PV covers the latency). They allocate from the
  pct pool (a QK-pair bank would still be waiting on its exp).
- ct leaves PSUM unnormalized right after the last PV (no z wait);
  normalization is an in-place SBUF mul off the critical path.
- Causal diagonal blocks stream only their unmasked columns; the z/exp
  pairing widens the two diagonal k-tiles of each chunk to their pair
  partner's width (band2-masked, both slots in one DVE mul), but PV
  still streams each k-tile's natural width.
- Softmax: z accumulated on the PE (fp8 DoubleRow over k-tile pairs,
  PSUM group per head); 1/z via reciprocal_approx_fast (DVE) +
  partition_broadcast (Pool); normalization fused into the PSUM->SBUF
  context copy.
- PSUM: phase 1 pmm bufs=5; phase 2 pmm2 2x[128,2,SC] for QK pairs
  (4 banks) + pct 3 (ct accumulators and out-proj groups) + z row.
- DMA: weights/activations fetched via grouped multi-dim descriptors on
  both HWDGE rings (sync+scalar) interleaved; wo on the sync ring only
  (the scalar queue feeds attention exps); output stored bf16, host
  sums partials in f32.
"""
import sys
import numpy as np

sys.path.insert(0, "/opt/trn_rl_repo")

import ml_dtypes

BF = ml_dtypes.bfloat16
F8 = ml_dtypes.float8_e4m3

B, S, E = 4, 2048, 2048
H, KS = 16, 128
HG = 8              # heads per core
D = HG * KS         # 1024 projected dims per core
ROT = 64
SC = 512            # s/q chunk
NSC = S // SC       # 4
NET = E // 128      # 16 e-tiles
NKT = S // 128      # 16 k-tiles
BAND_W = 896        # mask band table width
C_EXP = 2.0         # exp centering: pt = exp(logit - C_EXP), z cancels it

_PROG = {}


def _build_program(wbufs=2):
    import concourse.bass as bass
    import concourse.tile as tile
    import concourse.mybir as mybir
    from concourse import bacc
    from concourse.bass import ts, ds
    from contextlib import ExitStack

    f32 = mybir.dt.float32
    bf16 = mybir.dt.bfloat16
    fp8 = mybir.dt.float8e4
    AF = mybir.ActivationFunctionType
    DR = mybir.MatmulPerfMode.DoubleRow

    nc = bacc.Bacc("TRN2", target_bir_lowering=False, debug=False, num_devices=8)

    xq_d = nc.dram_tensor("xqT", [E, S], bf16, kind="ExternalInput").ap()
    xk_d = nc.dram_tensor("xkT", [E, S], bf16, kind="ExternalInput").ap()
    xv_d = nc.dram_tensor("xvT", [E, S], bf16, kind="ExternalInput").ap()
    wq_d = nc.dram_tensor("wq", [E, D], bf16, kind="ExternalInput").ap()
    wk_d = nc.dram_tensor("wk", [E, D], bf16, kind="ExternalInput").ap()
    wv_d = nc.dram_tensor("wv", [E, D], bf16, kind="ExternalInput").ap()
    wo_d = nc.dram_tensor("wo", [D, E], bf16, kind="ExternalInput").ap()
    ctab_d = nc.dram_tensor("ctab", [128, S], bf16, kind="ExternalInput").ap()
    stab_d = nc.dram_tensor("stab", [64, S], bf16, kind="ExternalInput").ap()
    band_d = nc.dram_tensor("band", [128, 2 * SC], fp8, kind="ExternalInput").ap()
    out_d = nc.dram_tensor("out", [S, E], bf16, kind="ExternalOutput").ap()

    with tile.TileContext(nc) as tc, ExitStack() as ctx:
        const = ctx.enter_context(tc.tile_pool(name="const", bufs=1))
        persist = ctx.enter_context(tc.tile_pool(name="persist", bufs=1))

        ctab = const.tile([128, S], bf16)
        stab = const.tile([64, S], bf16)
        band2 = const.tile([128, 2, SC], fp8)   # paired diag masks (ext 0/128)
        ones2 = const.tile([128, 2, 16], fp8)   # z DoubleRow stationary
        expb = const.tile([128, 1], f32)        # exp bias (-C_EXP)

        v_all = persist.tile([128, NKT, D], bf16)     # V[s, d] per k-tile
        k_all = persist.tile([128, HG, S], bf16)      # K^T[d, k] per head
        q_all = persist.tile([128, HG, S], bf16)      # Q^T[d, q] per head

        nc.vector.memset(ones2[:], 1.0)
        nc.vector.memset(expb[:], -C_EXP)

        # ---------------- Phase 1: projections (+RoPE for Q/K) ------------
        with tc.tile_pool(name="wpool", bufs=wbufs) as wpool, \
                tc.tile_pool(name="xtp", bufs=2) as xtp, \
                tc.tile_pool(name="rope", bufs=2) as rope, \
                tc.tile_pool(name="pmm", bufs=5, space="PSUM") as pmm:
            for proj, w_src, x_src in (("v", wv_d, xv_d), ("k", wk_d, xk_d),
                                       ("q", wq_d, xq_d)):
                w_sb = wpool.tile([128, NET, D], bf16, tag="wt")
                xt0 = xtp.tile([128, NET, SC], bf16, tag="xt")
                s0 = 3 if proj == "q" else 0      # first-emitted chunk
                if proj == "v":
                    # startup is DMA-latency-bound: fully fine-grained
                    # fetches keep the first matmul chain fed e-tile by
                    # e-tile as transfers land
                    for et in range(NET):
                        weng = nc.scalar if et % 2 == 0 else nc.sync
                        xeng = nc.sync if et % 2 == 0 else nc.scalar
                        weng.dma_start(w_sb[:, et, :],
                                       w_src[ts(et, 128), :])
                        xeng.dma_start(xt0[:, et, :],
                                       x_src[ts(et, 128), 0:SC])
                else:
                    # transition: fine-grained so the first chain starts fast
                    for et in range(NET):
                        weng = nc.scalar if et % 2 == 0 else nc.sync
                        xeng = nc.sync if et % 2 == 0 else nc.scalar
                        weng.dma_start(w_sb[:, et, :],
                                       w_src[ts(et, 128), :])
                        xeng.dma_start(xt0[:, et, :],
                                       x_src[ts(et, 128), ts(s0, SC)])
                if proj == "k":
                    # Deferred prefetch: tables are first needed by K's RoPE;
                    # keep them off the rings during V-proj.
                    nc.scalar.dma_start(ctab[:], ctab_d[:])
                    nc.scalar.dma_start(stab[:], stab_d[:])
                    nc.scalar.dma_start(band2[:], band_d[:])
                sc_order = [3, 2, 1, 0] if proj == "q" else [0, 1, 2, 3]
                for si, sc in enumerate(sc_order):
                    if si == 0:
                        xt = xt0
                    else:
                        xt = xtp.tile([128, NET, SC], bf16, tag="xt")
                        for eg in range(4):
                            eng = nc.sync if eg % 2 == 0 else nc.scalar
                            eng.dma_start(
                                xt[:, ts(eg, 4), :],
                                x_src[ts(eg, 512), ts(sc, SC)].rearrange(
                                    "(et p) s -> p et s", p=128))
                    if proj in ("q", "k"):
                        for h in range(HG):
                            ps = pmm.tile([128, SC], f32, tag="mm")
                            for et in range(NET):
                                nc.tensor.matmul(
                                    ps[:], w_sb[:, et, ts(h, 128)],
                                    xt[:, et, :],
                                    start=(et == 0), stop=(et == NET - 1))
                            qraw = rope.tile([128, SC], bf16, tag="qraw")
                            nc.vector.tensor_copy(qraw[:], ps[:])
                            qsw = rope.tile([64, SC], bf16, tag="qsw")
                            nc.sync.dma_start(qsw[0:32, :], qraw[32:64, :])
                            nc.sync.dma_start(qsw[32:64, :], qraw[0:32, :])
                            if proj == "k":
                                dst = k_all[:, h, ts(sc, SC)]
                            else:
                                dst = q_all[:, h, ts(sc, SC)]
                            nc.vector.tensor_mul(dst, qraw[:],
                                                 ctab[:, ts(sc, SC)])
                            t2 = rope.tile([64, SC], bf16, tag="t2")
                            nc.vector.tensor_mul(t2[:], qsw[:],
                                                 stab[:, ts(sc, SC)])
                            nc.vector.tensor_add(dst[0:64, :], dst[0:64, :],
                                                 t2[:])
                    else:
                        for ss in range(SC // 128):
                            for dc in range(D // SC):
                                ps = pmm.tile([128, SC], f32, tag="mm")
                                for et in range(NET):
                                    nc.tensor.matmul(
                                        ps[:], xt[:, et, ts(ss, 128)],
                                        w_sb[:, et, ts(dc, SC)],
                                        start=(et == 0), stop=(et == NET - 1))
                                nc.vector.tensor_copy(
                                    v_all[:, sc * 4 + ss, ts(dc, SC)], ps[:])

        # -------- Phase 2: fused attention + out-projection per q-chunk ---
        p2 = ExitStack()
        ctx.enter_context(p2)
        wop = p2.enter_context(tc.tile_pool(name="wop", bufs=1))
        ptp = p2.enter_context(tc.tile_pool(name="ptp", bufs=12))
        ctsbp = p2.enter_context(tc.tile_pool(name="ctsbp", bufs=2))
        zp = p2.enter_context(tc.tile_pool(name="zp", bufs=3))
        rbp = p2.enter_context(tc.tile_pool(name="rbp", bufs=3))
        osb = p2.enter_context(tc.tile_pool(name="osb", bufs=3))
        # PSUM: pmm2 holds QK pairs AND out-proj accumulators (4 banks),
        # pct the per-head context accumulators (3), pz the z rows (1).
        pmm2 = p2.enter_context(tc.tile_pool(name="pmm2", bufs=2,
                                             space="PSUM"))
        pct = p2.enter_context(tc.tile_pool(name="pct", bufs=3, space="PSUM"))
        pz = p2.enter_context(tc.tile_pool(name="pz", bufs=1, space="PSUM"))
        wo_sb = wop.tile([128, HG, E], bf16)          # Wo rows per head
        for g in range(HG):
            # sync ring only: the scalar queue feeds attention exps now
            nc.sync.dma_start(wo_sb[:, g, :], wo_d[ts(g, 128), :])
        zt = pz.tile([1, SC], f32)                    # z row (partition 0)

        def outproj_group(qc, ct_sb, qt, ec):
            # allocate from pct: avoids stealing a QK-pair bank whose exp
            # is still pending at head boundaries
            o_ps = pct.tile([128, SC], f32, tag="ctps")
            for hh in range(HG):
                nc.tensor.matmul(o_ps[:],
                                 ct_sb[:, hh, ts(qt, 128)],
                                 wo_sb[:, hh, ts(ec, SC)],
                                 start=(hh == 0), stop=(hh == HG - 1))
            o_sb = osb.tile([128, SC], bf16, tag="o")
            nc.vector.tensor_copy(o_sb[:], o_ps[:])
            eng = nc.sync if (qt + ec) % 2 == 0 else nc.scalar
            eng.dma_start(out_d[ts(qc * 4 + qt, 128), ts(ec, SC)], o_sb[:])

        def outproj_groups(qc, ct_sb):
            # 16 (qt, ec) groups; interleaved 2-per-head into the NEXT
            # chunk's attention so the PE absorbs the scalar exp rate and
            # the scalar queue never drains during out-projection.
            for qt in range(SC // 128):
                for ec in range(E // SC):
                    yield (qc, ct_sb, qt, ec)

        prev = None
        for qc in (3, 2, 1, 0):
            nkt = 4 * qc + 4
            ct_sb = ctsbp.tile([128, HG, SC], bf16, tag="ct")
            for h in range(HG):
                ct_ps = pct.tile([128, SC], f32, tag="ctps")
                z_ps = zt[:]
                for p in range(nkt // 2):
                    kt0 = 2 * p
                    off0 = kt0 * 128 - qc * SC
                    # pair window: both slots span [cs, cs+w)
                    cs = max(0, off0)
                    w = SC - cs
                    pt2 = ptp.tile([128, 2, SC], fp8, tag="pt")
                    l2 = pmm2.tile([128, 2, SC], f32, tag="mm2")
                    for j in range(2):
                        nc.tensor.matmul(l2[:, j, 0:w],
                                         k_all[:, h, ts(kt0 + j, 128)],
                                         q_all[:, h, ds(qc * SC + cs, w)],
                                         start=True, stop=True)
                    # one exp per pair: reads both PSUM banks, writes fp8
                    nc.scalar.activation(pt2[:, :, 0:w], l2[:, :, 0:w],
                                         AF.Exp, bias=expb[:])
                    if off0 + 128 > 0:
                        # diagonal pair: slots always mask at ext (0, 128)
                        nc.vector.tensor_mul(pt2[:, :, 0:w], pt2[:, :, 0:w],
                                             band2[:, :, 0:w])
                    if p == 0 and prev is not None:
                        # fill the head-start exp-latency bubble (the first
                        # PV waits ~1us on this pair's exp; mid-head the
                        # previous pair's PV covers it) with out-proj work
                        for _ in range(2):
                            g = next(prev, None)
                            if g is not None:
                                outproj_group(*g)
                    for j in range(2):
                        kt = kt0 + j
                        pcs = max(0, kt * 128 - qc * SC)
                        nc.tensor.matmul(
                            ct_ps[:, ds(pcs, SC - pcs)],
                            v_all[:, kt, ts(h, 128)],
                            pt2[:, j, ds(pcs - cs, SC - pcs)],
                            start=(kt == 0), stop=(kt == nkt - 1),
                            skip_group_check=True)
                    nc.tensor.matmul(z_ps[:, ds(cs, w)], ones2[:, :, 0:1],
                                     pt2[:, :, 0:w],
                                     start=(p == 0), stop=(p == nkt // 2 - 1),
                                     perf_mode=DR, skip_group_check=True)
                # Copy ct out UNNORMALIZED right after the last PV: frees the
                # PSUM bank without waiting the z->recip->broadcast chain
                # (otherwise the next head's first PV stalls ~0.8us on the
                # bank). Normalization happens in-place in SBUF off the
                # critical path; out-proj depends on the in-place mul.
                nc.vector.tensor_copy(ct_sb[:, h, :], ct_ps[:])
                zr = zp.tile([1, SC], f32, tag="zr")
                nc.vector.reciprocal_approx_fast(zr[:], z_ps)
                rb = rbp.tile([128, SC], f32, tag="rb")
                nc.gpsimd.partition_broadcast(rb[:], zr[:])
                nc.vector.tensor_mul(ct_sb[:, h, :], ct_sb[:, h, :], rb[:])
            prev = outproj_groups(qc, ct_sb)
        for g in prev:
            outproj_group(*g)

    nc.compile()
    return nc


def _get_program():
    if "nc" not in _PROG:
        try:
            _PROG["nc"] = _build_program(wbufs=2)
        except Exception:
            _PROG["nc"] = _build_program(wbufs=1)
    return _PROG["nc"]


def _host_prep(query_inputs, key_inputs, value_inputs, Wq, Wk, Wv, Wo):
    """Shard + bf16-cast inputs; fold scale/permutation into Wq/Wk."""
    perm = np.concatenate([np.arange(0, ROT, 2), np.arange(1, ROT, 2),
                           np.arange(ROT, KS)])
    Wq_p = (Wq.reshape(E, H, KS)[:, :, perm] / np.float32(np.sqrt(KS))
            ).reshape(E, H * KS)
    Wk_p = Wk.reshape(E, H, KS)[:, :, perm].reshape(E, H * KS)

    inv_freq = 1.0 / (10000.0 ** (np.arange(0, ROT, 2, dtype=np.float32) / ROT))
    ang = np.outer(np.arange(S, dtype=np.float32), inv_freq)  # (S, 32)
    sin = np.sin(ang).T.astype(np.float32)
    cos = np.cos(ang).T.astype(np.float32)
    ctab = np.ones((128, S), np.float32)
    ctab[0:32] = cos
    ctab[32:64] = cos
    stab = np.zeros((64, S), np.float32)
    stab[0:32] = -sin
    stab[32:64] = sin
    # band[i, c] = 1 iff (c - 384) >= i. Diagonal pairs always mask their
    # two k-tile slots at ext (0, 128): band2 slot0 = band[:, 384:896],
    # slot1 = band[:, 256:768], each 512 wide (use first w cols).
    cgrid = np.arange(BAND_W)[None, :] - 384
    band = (cgrid >= np.arange(128)[:, None]).astype(np.float32)
    band2 = np.concatenate([band[:, 384:896], band[:, 256:768]], axis=1)

    shared = {
        "ctab": ctab.astype(BF),
        "stab": stab.astype(BF),
        "band": band2.astype(F8),
    }
    in_maps = []
    for c in range(8):
        b, g = c // 2, c % 2
        cols = slice(g * D, (g + 1) * D)
        in_maps.append({
            "xqT": np.ascontiguousarray(query_inputs[b].T).astype(BF),
            "xkT": np.ascontiguousarray(key_inputs[b].T).astype(BF),
            "xvT": np.ascontiguousarray(value_inputs[b].T).astype(BF),
            "wq": np.ascontiguousarray(Wq_p[:, cols]).astype(BF),
            "wk": np.ascontiguousarray(Wk_p[:, cols]).astype(BF),
            "wv": np.ascontiguousarray(Wv[:, cols]).astype(BF),
            "wo": np.ascontiguousarray(Wo[cols, :]).astype(BF),
            **shared,
        })
    return in_maps


def run_sharded(inputs, trace=False, **trace_kw):
    """Build + run the SPMD kernel; returns (output, BassKernelResults)."""
    from concourse.bass_utils import run_bass_kernel_spmd

    nc = _get_program()
    in_maps = _host_prep(
        np.asarray(inputs["query_inputs"], np.float32),
        np.asarray(inputs["key_inputs"], np.float32),
        np.asarray(inputs["value_inputs"], np.float32),
        np.asarray(inputs["Wq"], np.float32),
        np.asarray(inputs["Wk"], np.float32),
        np.asarray(inputs["Wv"], np.float32),
        np.asarray(inputs["Wo"], np.float32),
    )
    br = run_bass_kernel_spmd(nc, in_maps, list(range(8)), trace=trace,
                              **trace_kw)
    parts = [np.asarray(r["out"], np.float32) for r in br.results]
    out = np.stack([parts[2 * b] + parts[2 * b + 1] for b in range(B)])
    return out, br


def kernel(query_inputs, key_inputs, value_inputs, attention_mask,
           Wq, Wk, Wv, Wo):
    out, _ = run_sharded({
        "query_inputs": query_inputs, "key_inputs": key_inputs,
        "value_inputs": value_inputs, "Wq": Wq, "Wk": Wk, "Wv": Wv, "Wo": Wo,
    })
    return out
